# revision 6
# baseline (speedup 1.0000x reference)
"""Trainium2 Bass kernel for nn_Attention (sparse_attention variant).

Computes, for inputs hidden/encoder_outputs [B,S,D], c_t [B,D], W [OUT,3D],
b [OUT], v [OUT]:

    cat       = concat([hidden, broadcast(c_t), encoder_outputs], axis=2)
    energy    = relu(cat @ W.T + b)            # [B, S, OUT]
    attention = energy @ v                     # [B, S]
    out       = softmax(attention, axis=1)

Strategy (8 NeuronCores, data-parallel over batch, 2 batches/core):
  - Split W = [W1 | W2 | W3] over the feature axis.  Since v >= 0,
    relu(z)*v == relu(z*v), so pre-scale W'=W*v[:,None], b'=b*v and the
    v-dot becomes a plain row-sum of relu(pre-activations).
  - c2[b,:] = c_t[b] @ W2'.T + b' is computed once per batch and injected
    into each PSUM accumulation via a rank-1 (K=1) ones matmul.
  - Main loop per 128-row s-tile: PE-transpose X tiles (contraction dim f
    must sit on partitions), then accumulate
        pre[s, o] = X^T-tiles.T @ W'^T-tiles  (+ c2 broadcast)
    over 16 f-chunks x 2 PSUM banks, then one ScalarE pass does
    relu + free-dim row-sum (accum_out) -> attention logits.
  - Softmax over S=2048 per batch: 128x16 tile, DVE free-dim reduce +
    GpSimd partition all-reduce, ScalarE exp, DVE normalize.
"""

import sys
import numpy as np

for _p in ("/opt/trn_rl_repo",):
    if _p not in sys.path:
        sys.path.insert(0, _p)

import concourse.bass as bass
import concourse.bacc as bacc
import concourse.tile as tile
from concourse import mybir, bass_isa
from concourse.bass_utils import run_bass_kernel_spmd
from concourse.masks import make_identity

F32 = mybir.dt.float32
F32R = mybir.dt.float32r
BF16 = mybir.dt.bfloat16
AF = mybir.ActivationFunctionType

B, S, D, OUT = 16, 2048, 1024, 1024
N_CORES = 8
B_LOC = B // N_CORES            # batches per core
S_LOC = B_LOC * S               # 4096 rows of X per core
N_ST = S_LOC // 128             # 32 s-tiles per core
ST_PER_B = S // 128             # 16 s-tiles per batch
FC = D // 128                   # 8 feature chunks per tensor
NB = OUT // 512                 # 2 PSUM banks across OUT

MM_MODE = "f32r"                # "f32r" | "f32"


def _mm(ap):
    """View a float32 AP as the matmul dtype (consumer side)."""
    if MM_MODE == "f32r":
        return ap.bitcast(F32R)
    return ap


def _rnd(ap):
    """Producer-side view: writes through this AP round to the matmul grid."""
    if MM_MODE == "f32r":
        return ap.bitcast(F32R)
    return ap


def build_nc():
    nc = bacc.Bacc("TRN2", target_bir_lowering=False, debug=False,
                   num_devices=N_CORES)

    hid = nc.dram_tensor("hidden", [S_LOC, D], F32, kind="ExternalInput").ap()
    enc = nc.dram_tensor("enc", [S_LOC, D], F32, kind="ExternalInput").ap()
    ct = nc.dram_tensor("ct", [B_LOC, D], F32, kind="ExternalInput").ap()
    Wd = nc.dram_tensor("W", [OUT, 3 * D], F32, kind="ExternalInput").ap()
    bd = nc.dram_tensor("b", [OUT], F32, kind="ExternalInput").ap()
    vd = nc.dram_tensor("v", [OUT], F32, kind="ExternalInput").ap()
    outd = nc.dram_tensor("out", [B_LOC, S], F32, kind="ExternalOutput").ap()

    with tile.TileContext(nc) as tc:
        with (
            tc.tile_pool(name="const", bufs=1) as cpool,
            tc.tile_pool(name="wT", bufs=1) as wpool,
            tc.tile_pool(name="wload", bufs=2) as wload,
            tc.tile_pool(name="x", bufs=3) as xpool,
            tc.tile_pool(name="xT", bufs=2) as xTpool,
            tc.tile_pool(name="scratch", bufs=2) as spool,
            tc.tile_pool(name="sm", bufs=2) as smpool,
            tc.tile_pool(name="ptp", bufs=3, space=bass.MemorySpace.PSUM) as ptp,
            tc.tile_pool(name="eps", bufs=2, space=bass.MemorySpace.PSUM) as eps,
        ):
            # ---- constants -------------------------------------------------
            ident = cpool.tile([128, 128], F32)
            make_identity(nc, ident[:])

            ones_f = cpool.tile([1, 128], F32)
            nc.vector.memset(ones_f[:], 1.0)
            ones_k1 = cpool.tile([1, 128], F32)
            nc.vector.tensor_copy(_rnd(ones_k1[:]), ones_f[:])

            v_sb = cpool.tile([128, FC], F32)       # v[oc*128+p] -> [p, oc]
            nc.sync.dma_start(v_sb[:], vd.rearrange("(oc p) -> p oc", p=128))

            b_row = cpool.tile([1, OUT], F32)
            nc.sync.dma_start(b_row[:], bd[None, :])
            v_row = cpool.tile([1, OUT], F32)
            nc.sync.dma_start(v_row[:], vd[None, :])
            bv = cpool.tile([1, OUT], F32)          # b' = b * v
            nc.vector.tensor_mul(_rnd(bv[:]), b_row[:], v_row[:])

            ctT = cpool.tile([128, FC, B_LOC], F32)  # ct[b, fc*128+p] -> [p,fc,b]
            for bb in range(B_LOC):
                nc.sync.dma_start(ctT[:, :, bb],
                                  ct[bb].rearrange("(fc p) -> p fc", p=128))
            ctT_r = cpool.tile([128, FC, B_LOC], F32)
            nc.vector.tensor_copy(_rnd(ctT_r[:]), ctT[:])

            att_all = cpool.tile([128, N_ST], F32)   # attention logits

            # ---- W' = W*v, transposed to [f-part, o-free] ------------------
            # wT[:, j, :] holds chunk j of W'^T: j in [0,8)=W1, [8,16)=W2,
            # [16,24)=W3; entry [p, j, o] = W[o, j*128+p] * v[o].
            wT = wpool.tile([128, 3 * FC, OUT], F32)
            for oc in range(FC):
                w_nat = wload.tile([128, 3 * D], F32)
                nc.sync.dma_start(w_nat[:], Wd[oc * 128:(oc + 1) * 128, :])
                nc.vector.tensor_scalar_mul(w_nat[:], w_nat[:], v_sb[:, oc:oc + 1])
                for j in range(3 * FC):
                    pt = ptp.tile([128, 128], F32, tag="tp")
                    nc.tensor.transpose(pt[:], w_nat[:, j * 128:(j + 1) * 128],
                                        ident[:])
                    nc.vector.tensor_copy(_rnd(wT[:, j, oc * 128:(oc + 1) * 128]), pt[:])

            # ---- c2[b,:] = c_t[b] @ W2'.T + b' ----------------------------
            c2_sb = []
            for bb in range(B_LOC):
                c2_ps = eps.tile([1, OUT], F32, tag="eps")
                for ob in range(NB):
                    sl = slice(ob * 512, (ob + 1) * 512)
                    for fc in range(FC):
                        nc.tensor.matmul(c2_ps[:, sl],
                                         _mm(ctT_r[:, fc, bb:bb + 1]),
                                         _mm(wT[:, FC + fc, sl]),
                                         start=(fc == 0), stop=False)
                    nc.tensor.matmul(c2_ps[:, sl], _mm(ones_k1[:, :1]),
                                     _mm(bv[:, sl]), start=False, stop=True)
                c2b = cpool.tile([1, OUT], F32, tag=f"c2_{bb}")
                nc.vector.tensor_copy(_rnd(c2b[:]), c2_ps[:])
                c2_sb.append(c2b)

            # ---- main loop over s-tiles -----------------------------------
            for st in range(N_ST):
                b_idx = st // ST_PER_B
                rows = slice(st * 128, (st + 1) * 128)

                x_h = xpool.tile([128, D], F32, tag="xh")
                nc.sync.dma_start(x_h[:], hid[rows, :])
                x_e = xpool.tile([128, D], F32, tag="xe")
                nc.sync.dma_start(x_e[:], enc[rows, :])

                hT = xTpool.tile([128, FC, 128], F32, tag="hT")
                eT = xTpool.tile([128, FC, 128], F32, tag="eT")
                for fc in range(FC):
                    pt = ptp.tile([128, 128], F32, tag="tp")
                    nc.tensor.transpose(pt[:], x_h[:, fc * 128:(fc + 1) * 128],
                                        ident[:])
                    nc.vector.tensor_copy(_rnd(hT[:, fc, :]), pt[:])
                    pt2 = ptp.tile([128, 128], F32, tag="tp")
                    nc.tensor.transpose(pt2[:], x_e[:, fc * 128:(fc + 1) * 128],
                                        ident[:])
                    nc.vector.tensor_copy(_rnd(eT[:, fc, :]), pt2[:])

                e_ps = eps.tile([128, OUT], F32, tag="eps")
                for ob in range(NB):
                    sl = slice(ob * 512, (ob + 1) * 512)
                    # rank-1 broadcast of c2[b] across all 128 partitions
                    nc.tensor.matmul(e_ps[:, sl], _mm(ones_k1[:]),
                                     _mm(c2_sb[b_idx][:, sl]),
                                     start=True, stop=False)
                    for fc in range(FC):
                        nc.tensor.matmul(e_ps[:, sl], _mm(hT[:, fc, :]),
                                         _mm(wT[:, fc, sl]),
                                         start=False, stop=False)
                    for fc in range(FC):
                        nc.tensor.matmul(e_ps[:, sl], _mm(eT[:, fc, :]),
                                         _mm(wT[:, 2 * FC + fc, sl]),
                                         start=False, stop=(fc == FC - 1))

                relu_out = spool.tile([128, OUT], BF16, tag="relu")
                nc.scalar.activation(relu_out[:], e_ps[:], AF.Relu,
                                     accum_out=att_all[:, st:st + 1])

            # ---- per-batch softmax over S ---------------------------------
            for bb in range(B_LOC):
                sl = slice(bb * ST_PER_B, (bb + 1) * ST_PER_B)
                m1 = smpool.tile([128, 1], F32, tag="m1")
                nc.vector.tensor_reduce(m1[:], att_all[:, sl],
                                        axis=mybir.AxisListType.X,
                                        op=mybir.AluOpType.max)
                mall = smpool.tile([128, 1], F32, tag="mall")
                nc.gpsimd.partition_all_reduce(mall[:], m1[:], channels=128,
                                               reduce_op=bass_isa.ReduceOp.max)
                nmall = smpool.tile([128, 1], F32, tag="nmall")
                nc.vector.tensor_scalar_mul(nmall[:], mall[:], -1.0)
                ex = smpool.tile([128, ST_PER_B], F32, tag="ex")
                nc.scalar.activation(ex[:], att_all[:, sl], AF.Exp,
                                     bias=nmall[:])
                rs = smpool.tile([128, 1], F32, tag="rs")
                nc.vector.tensor_reduce(rs[:], ex[:],
                                        axis=mybir.AxisListType.X,
                                        op=mybir.AluOpType.add)
                tot = smpool.tile([128, 1], F32, tag="tot")
                nc.gpsimd.partition_all_reduce(tot[:], rs[:], channels=128,
                                               reduce_op=bass_isa.ReduceOp.add)
                rec = smpool.tile([128, 1], F32, tag="rec")
                nc.vector.reciprocal(rec[:], tot[:])
                res_t = smpool.tile([128, ST_PER_B], F32, tag="res")
                nc.vector.tensor_scalar_mul(res_t[:], ex[:], rec[:])
                nc.sync.dma_start(
                    outd[bb].rearrange("(stl p) -> p stl", p=128), res_t[:])

    nc.compile()
    return nc


_NC = None


def _get_nc():
    global _NC
    if _NC is None:
        _NC = build_nc()
    return _NC


def _in_maps(hidden, encoder_outputs, c_t, W, b, v):
    hidden = np.ascontiguousarray(hidden, dtype=np.float32)
    encoder_outputs = np.ascontiguousarray(encoder_outputs, dtype=np.float32)
    c_t = np.ascontiguousarray(c_t, dtype=np.float32)
    W = np.ascontiguousarray(W, dtype=np.float32)
    b = np.ascontiguousarray(b, dtype=np.float32)
    v = np.ascontiguousarray(v, dtype=np.float32)
    maps = []
    for i in range(N_CORES):
        bs = slice(i * B_LOC, (i + 1) * B_LOC)
        maps.append({
            "hidden": hidden[bs].reshape(S_LOC, D),
            "enc": encoder_outputs[bs].reshape(S_LOC, D),
            "ct": c_t[bs],
            "W": W, "b": b, "v": v,
        })
    return maps


def run(hidden, encoder_outputs, c_t, W, b, v, trace=False, tmpdir=None):
    nc = _get_nc()
    maps = _in_maps(hidden, encoder_outputs, c_t, W, b, v)
    res = run_bass_kernel_spmd(nc, maps, list(range(N_CORES)), trace=trace,
                               tmpdir=tmpdir)
    out = np.concatenate([res.results[i]["out"] for i in range(N_CORES)],
                         axis=0)
    return out, res


def kernel(hidden, encoder_outputs, c_t, W, b, v):
    out, _ = run(hidden, encoder_outputs, c_t, W, b, v)
    return out
